# revision 24
# baseline (speedup 1.0000x reference)
"""GAT layer Bass kernel for Trainium2, 8-core SPMD.

Sharding: core c handles batch b = c//2 and row-half ih = c%2 (512 rows of i).
Each core reads its 32MB slice of edge_fts once (memory-bound roofline).

Per-core layout trick: edge slab for row i is the contiguous 64KB region
E[i] viewed as [j_hi=128, (j_lo=8, e=16)=128].  A bf16 cast-load (SWDGE)
followed by an SBUF->SBUF DMA-transpose yields T_i[(j_lo,e), j_hi], and a
single matmul against a block-diagonal ae_w produces
att_e[j_hi, (h, j_lo)] with j (= j_hi*8+j_lo) on partitions -- the layout
softmax-over-j and the attention*V contraction both want.
"""
import sys
sys.path.insert(0, "/opt/trn_rl_repo")
from contextlib import ExitStack

import numpy as np

import concourse.bass as bass
import concourse.tile as tile
from concourse import mybir
from concourse.masks import make_identity

F32 = mybir.dt.float32
F32R = mybir.dt.float32r
BF16 = mybir.dt.bfloat16
AF = mybir.ActivationFunctionType
OP = mybir.AluOpType

B, N = 4, 1024
FN, FH, FE, FG = 128, 128, 16, 128
OUT, H = 128, 8
DH = OUT // H          # 16
ZIN = FN + FH          # 256
NC = 8                 # cores
NI = N // 2            # own rows per core = 512
NJH, NJL = N // 8, 8   # j = j_hi*8 + j_lo
NBLK = NI // 128       # i-blocks per core = 4
NOCT = 128 // 8        # octets per block = 16


def build_core_program(nc, n_iters=1):
    """Emit the full per-core program. Returns nothing; declares DRAM params:
    inputs: e_own[512,128,128]f32, nf[1024,128]f32, hd[1024,128]f32,
            gf[128,1]f32, adjb[128,4096]f32 (bias (adj-1)*1e9 in
            [j_hi, (i512, j_lo8)] layout),
            m_w[256,128], m_b[1,128], skip_w[256,128], skip_b[1,128],
            a1_w[256,8], a1_b[1,8], a2_w[256,8], a2_b[1,8],
            ae_w[16,8], ae_b[1,8], ag_w[128,8], ag_b[1,8]
    output: ret[512,128]f32
    """
    d = {}
    def inp(name, shape, dtype=F32):
        d[name] = nc.dram_tensor(name, shape, dtype, kind="ExternalInput").ap()
    inp("e_own", [NI // 8, 128, 1024], BF16)   # [oct, (j_lo,e), (i8, j_hi)]
    inp("nf", [N, FN])
    inp("hd", [N, FH])
    inp("gf", [FG, 1])
    inp("adjm", [NBLK, 128, 1024], BF16)       # [blk, j_hi, (i128, j_lo8)] 0/1
    inp("m_w", [ZIN, OUT]); inp("m_b", [1, OUT])
    inp("skip_w", [ZIN, OUT]); inp("skip_b", [1, OUT])
    inp("a1_w", [ZIN, H]); inp("a1_b", [1, H])
    inp("a2_w", [ZIN, H]); inp("a2_b", [1, H])
    inp("ae_w", [FE, H]); inp("ae_b", [1, H])
    inp("ag_w", [FG, H]); inp("ag_b", [1, H])
    ret = nc.dram_tensor("ret", [NI, OUT], F32, kind="ExternalOutput").ap()

    with tile.TileContext(nc) as tc:
        with ExitStack() as ctx:
            emit(ctx, tc, d, ret, n_iters)


def emit(ctx, tc, d, ret, n_iters):
    nc = tc.nc
    P = lambda name, bufs=1: ctx.enter_context(tc.tile_pool(name=name, bufs=bufs))
    PS = lambda name, bufs=1: ctx.enter_context(
        tc.tile_pool(name=name, bufs=bufs, space="PSUM"))

    const = P("const")          # long-lived constants / staging
    psc_pool = PS("ps_small", bufs=2)   # all [128, <=128] psum tiles, shared slots
    psmisc = PS("ps_misc", bufs=1)      # odd-shaped psum tiles
    class _PS:
        def tile(self, shape, dtype):
            return psc_pool.tile(shape, dtype, tag="s", name="pstile")
    psc = _PS()
    # ---------------- prologue: constants ----------------
    ident = const.tile([128, 128], F32)
    make_identity(nc, ident[:])
    ones_bf = const.tile([128, 128], BF16)
    nc.gpsimd.memset(ones_bf[:], 1.0)
    ones_row = const.tile([1, 128], F32)
    nc.gpsimd.memset(ones_row[:], 1.0)

    # small weights into sbuf; ZIN-row weights stored as [128, (2, n)]
    wpool = P("weights")
    def load(name, shape, dtype=F32):
        t = wpool.tile(shape, dtype, name=name)
        nc.gpsimd.dma_start(t[:], d[name][:])
        return t
    def load2(name, ncols):
        t = wpool.tile([128, 2, ncols], F32, name=name)
        nc.gpsimd.dma_start(t[:], d[name][:].rearrange("(c p) n -> p c n", c=2))
        return lambda ct: t[:, ct, :]
    m_w = load2("m_w", OUT);  m_b = load("m_b", [1, OUT])
    sk_w = load2("skip_w", OUT); sk_b = load("skip_b", [1, OUT])
    a1_w = load2("a1_w", H); a1_b = load("a1_b", [1, H])
    a2_w = load2("a2_w", H); a2_b = load("a2_b", [1, H])
    ae_w = load("ae_w", [FE, H]); ae_b = load("ae_b", [1, H])
    ag_w = load("ag_w", [FG, H]); ag_b = load("ag_b", [1, H])
    gf = load("gf", [FG, 1])
    import os
    K_STAGE = int(os.environ.get("K_STAGE", 99))
    if K_STAGE <= 1: return

    # blockdiag bd[(j_lo,e), (j_lo', h)] = ae_w[e,h] * (j_lo == j_lo')
    bd = const.tile([128, 64], BF16)
    nc.gpsimd.memset(bd[:], 0.0)
    ae_w_bf = wpool.tile([FE, H], BF16, name="ae_w_bf")
    nc.vector.tensor_copy(ae_w_bf[:], ae_w[:])
    for jl in range(NJL):
        # rows jl*16..jl*16+16, cols jl*8..jl*8+8 (j_lo-major col order)
        dst = bd[:].rearrange("p (j h) -> p j h", j=NJL)[jl * 16:(jl + 1) * 16, jl, :]
        nc.gpsimd.dma_start(dst, ae_w_bf[:, :])

    if K_STAGE <= 2: return
    # I64 identity bf16 (for att2g selector matmul)
    i64 = const.tile([64, 64], BF16)
    make_identity(nc, i64[:])

    # zT: [c, j] two c-tiles of [128, 1024] f32
    zT = const.tile([128, 2 * N], F32)  # cols [0:1024] = nf.T, [1024:2048] = hd.T
    zpool = P("zstage", bufs=3)
    for half, src in ((0, d["nf"]), (1, d["hd"])):
        for jb in range(N // 128):
            st = zpool.tile([128, 128], F32)
            nc.gpsimd.dma_start(st[:], src[jb * 128:(jb + 1) * 128, :])
            tp = psc.tile([128, 128], F32)
            nc.tensor.transpose(tp[:], st[:], ident[:])
            nc.vector.tensor_copy(
                zT[:, half * N + jb * 128: half * N + (jb + 1) * 128], tp[:])

    if K_STAGE <= 3: return
    def zT_half(h_idx):
        return zT[:, h_idx * N:(h_idx + 1) * N]

    # cst[h] = a1_b + a2_b + ae_b + ag_b + gf @ ag_w   (shape [1, 8])
    attg_ps = psmisc.tile([1, H], F32, tag="m", name="attg_ps")
    nc.tensor.matmul(attg_ps[:], gf[:], ag_w[:],
                     start=True, stop=True)
    if K_STAGE <= 31: return
    cstv = const.tile([1, H], F32)
    nc.vector.scalar_tensor_tensor(cstv[:], a1_b[:], 1.0, a2_b[:], OP.mult, OP.add)
    nc.vector.scalar_tensor_tensor(cstv[:], cstv[:], 1.0, ae_b[:], OP.mult, OP.add)
    nc.vector.scalar_tensor_tensor(cstv[:], cstv[:], 1.0, ag_b[:], OP.mult, OP.add)
    nc.vector.scalar_tensor_tensor(cstv[:], cstv[:], 1.0, attg_ps[:], OP.mult, OP.add)
    if K_STAGE <= 32: return
    # broadcast cst to all 128 partitions: ones_row.T @ cstv
    cstb_ps = psmisc.tile([128, H], F32, tag="m", name="cstb_ps")
    nc.tensor.matmul(cstb_ps[:], ones_row[:], cstv[:],
                     start=True, stop=True)
    cstb = const.tile([128, H], F32)
    nc.vector.tensor_copy(cstb[:], cstb_ps[:])

    if K_STAGE <= 4: return
    # att2g[j_hi, (j_lo, h)] = att_2[j, h] + cst[h];  att2gT bf16 [64, 128]
    att2g = const.tile([128, 64], F32)
    for jl in range(NJL):
        a2ps = psc.tile([128, H], F32)
        for ct in range(2):
            lhs = zT_half(ct)[:].rearrange("p (j l) -> p j l", l=8)[:, :, jl]
            nc.tensor.matmul(a2ps[:], lhs,
                             a2_w(ct),
                             start=(ct == 0), stop=(ct == 1))
        dst = att2g[:].rearrange("p (j h) -> p j h", j=NJL)[:, jl, :]
        nc.vector.scalar_tensor_tensor(dst, a2ps[:], 1.0, cstb[:], OP.mult, OP.add)
    att2gT_ps = psc.tile([64, 128], F32)
    nc.tensor.transpose(att2gT_ps[:], att2g[:], ident[:])
    att2gT = const.tile([64, 128], BF16)
    nc.vector.tensor_copy(att2gT[:], att2gT_ps[:])

    if K_STAGE <= 5: return
    # q_sum[k, (i, h)] bf16: folded z (x) a1_w product so that
    # ones.T @ q_sum = att_1[i, h] broadcast over j_hi partitions.
    q_sum = const.tile([128, NI * H], BF16)
    qtmp = const.tile([128, NI * H], F32)
    # q = zT[c, own_i] * a1w[c, h]; own rows are always zT columns 0..511
    for ct in range(2):
        zslice = zT_half(ct)[:, OWN_I0:OWN_I0 + NI]
        z3 = zslice.rearrange("p (i x) -> p i x", x=1).broadcast_to([128, NI, H])
        a3 = a1_w(ct).rearrange("p (x h) -> p x h", x=1).broadcast_to([128, NI, H])
        if ct == 0:
            nc.vector.scalar_tensor_tensor(
                qtmp[:].rearrange("p (i h) -> p i h", h=H), z3, 1.0, a3,
                OP.mult, OP.mult)
        else:
            q2 = const.tile([128, NI * H], F32)
            nc.vector.scalar_tensor_tensor(
                q2[:].rearrange("p (i h) -> p i h", h=H), z3, 1.0, a3,
                OP.mult, OP.mult)
            nc.vector.scalar_tensor_tensor(
                q_sum[:].rearrange("p (i h) -> p i h", h=H),
                qtmp[:].rearrange("p (i h) -> p i h", h=H), 1.0,
                q2[:].rearrange("p (i h) -> p i h", h=H), OP.mult, OP.add)

    if K_STAGE <= 6: return
    # V_perm[j_hi, (h, j_lo, 17)] bf16; col 16 of each (h,j_lo) group is 1.0
    v_perm = const.tile([128, H * NJL * (DH + 1)], BF16)
    nc.gpsimd.memset(v_perm[:], 1.0)
    m_b_bc_ps = psc.tile([128, OUT], F32)
    nc.tensor.matmul(m_b_bc_ps[:], ones_row[:], m_b[:],
                     start=True, stop=True)
    m_b_bc = const.tile([128, OUT], F32)
    nc.vector.tensor_copy(m_b_bc[:], m_b_bc_ps[:])
    for jl in range(NJL):
        vps = psc.tile([128, OUT], F32)
        for ct in range(2):
            lhs = zT_half(ct)[:].rearrange("p (j l) -> p j l", l=8)[:, :, jl]
            nc.tensor.matmul(vps[:], lhs,
                             m_w(ct),
                             start=(ct == 0), stop=(ct == 1))
        dst = v_perm[:].rearrange("p (h j d) -> p h j d", h=H, j=NJL)[:, :, jl, 0:DH]
        nc.vector.scalar_tensor_tensor(
            dst, vps[:].rearrange("p (h d) -> p h d", h=H), 1.0,
            m_b_bc[:].rearrange("p (h d) -> p h d", h=H), OP.mult, OP.add)

    if K_STAGE <= 7: return
    # skip_b broadcast
    skb_ps = psc.tile([128, OUT], F32)
    nc.tensor.matmul(skb_ps[:], ones_row[:], sk_b[:],
                     start=True, stop=True)
    skb = const.tile([128, OUT], F32)
    nc.vector.tensor_copy(skb[:], skb_ps[:])

    # ---------------- main loop ----------------
    slabp = P("slab", bufs=6)
    maskp = P("maskb", bufs=2)
    lp = PS("logits", bufs=4)
    pblk = P("pblock", bufs=3)
    rp = P("rasm", bufs=2)
    outp = P("outs", bufs=2)

    import os
    nblk = int(os.environ.get("K_NBLK", NBLK))
    noct = int(os.environ.get("K_NOCT", NOCT))
    for it in range(n_iters):
        for ib in range(nblk):
            if DMA_ONLY:
                ret_probe = rp.tile([128, NOCT], F32, name="ret_probe")
            mblk = maskp.tile([128, 1024], BF16)
            nc.gpsimd.dma_start(mblk[:], d["adjm"][ib])
            p_block = pblk.tile([128, 128 * 64], BF16)  # (i 128, h 8, j_lo 8)
            for oct in range(noct):
                i0 = ib * 128 + oct * 8  # own-row index of first i in octet
                # bf16 slab, pre-transposed on host: [(j_lo,e), (i8, j_hi)]
                if not COMPUTE_ONLY or oct == 0:
                    s8 = slabp.tile([128, 1024], BF16)
                    nc.sync.dma_start(s8[:], d["e_own"][ib * NOCT + oct])
                if DMA_ONLY:
                    nc.vector.tensor_copy(
                        ret_probe[:, oct:oct + 1], s8[:, 0:1])
                    continue
                L = lp.tile([128, 512], F32)
                # col order: (i 8, j_lo 8, h 8)
                L4 = L[:].rearrange("p (i j h) -> p i j h", i=8, j=NJL)
                # att_1 (+ j_lo broadcast) : ones.T @ q_sum-slice
                qsl = (q_sum[:].rearrange("p (i x h) -> p i x h", x=1, h=H)
                       [:, i0:i0 + 8].broadcast_to([128, 8, NJL, H]))
                nc.tensor.matmul(L4, ones_bf[:], qsl,
                                 start=True, stop=False, skip_group_check=True)
                # att_2 + cst : att2gT.T @ I64 (broadcast over i)
                isel = i64[:].rearrange("p (x n) -> p x n", x=1).broadcast_to([64, 8, 64])
                nc.tensor.matmul(L4.rearrange("p i j h -> p i (j h)"), att2gT[:],
                                 isel, start=False, stop=False, skip_group_check=True)
                # att_e per i
                for il in range(8):
                    lhs = s8[:, il * 128:(il + 1) * 128]
                    nc.tensor.matmul(L[:, il * 64:(il + 1) * 64], lhs, bd[:],
                                     start=False, stop=(il == 7),
                                     skip_group_check=True)
                # leaky relu in place (PSUM)
                nc.scalar.activation(L[:], L[:], AF.Prelu, alpha=0.01)
                # exp -> bf16 into P block
                pslice = p_block[:, oct * 512:(oct + 1) * 512]
                nc.scalar.activation(pslice, L[:], AF.Exp)
            # adjacency mask for whole block: multiply by 0/1 mask (h last)
            pv = p_block[:].rearrange("p (f h) -> p f h", h=H)
            mv = (mblk[:].rearrange("p (f x) -> p f x", x=1)
                  .broadcast_to([128, 1024, H]))
            nc.gpsimd.tensor_tensor(pv, pv, mv, OP.mult)

            if DMA_ONLY:
                nc.gpsimd.dma_start(ret[ib * 128:(ib + 1) * 128, 0:NOCT],
                                    ret_probe[:])
                continue
            # attention @ V for this block
            r_asm = rp.tile([128, OUT], F32)
            pb4 = p_block[:].rearrange("p (i j h) -> p i j h", i=128, j=NJL)
            vp4 = v_perm[:].rearrange("p (h j d) -> p h j d", h=H, j=NJL)
            for h in range(H):
                av = psc.tile([128, DH + 1], F32)
                for jl in range(NJL):
                    nc.tensor.matmul(av[:], pb4[:, :, jl, h], vp4[:, h, jl, :],
                                     start=(jl == 0), stop=(jl == 7))
                recip = rp.tile([128, 1], F32)
                nc.vector.reciprocal(recip[:], av[:, DH:DH + 1])
                nc.vector.tensor_scalar_mul(
                    r_asm[:, h * DH:(h + 1) * DH], av[:, 0:DH], recip[:])

            # skip connection + relu + store
            sk = psc.tile([128, OUT], F32)
            for ct in range(2):
                lhs = zT_half(ct)[:, OWN_I0 + ib * 128:OWN_I0 + (ib + 1) * 128]
                nc.tensor.matmul(sk[:], lhs,
                                 sk_w(ct),
                                 start=(ct == 0), stop=False,
                                 skip_group_check=True)
            nc.tensor.matmul(sk[:], ones_row[:],
                             sk_b[:], start=False, stop=True,
                             skip_group_check=True)
            nc.vector.scalar_tensor_tensor(sk[:], sk[:], 1.0, r_asm[:],
                                           OP.mult, OP.add)
            ob = outp.tile([128, OUT], F32)
            nc.scalar.activation(ob[:], sk[:], AF.Relu)
            nc.gpsimd.dma_start(ret[ib * 128:(ib + 1) * 128, :], ob[:])


import os as _os
OWN_I0 = 0  # own rows always at z columns 0..511 (inputs pre-rotated)
USE_PRELU = True  # ACT parametric relu on HW; test_sim sets False
DMA_ONLY = _os.environ.get("K_DMA_ONLY", "0") == "1"
COMPUTE_ONLY = _os.environ.get("K_COMPUTE_ONLY", "0") == "1"


def split_multi_waits(nc):
    """Walrus codegen limits sem-waits per instruction (1 on Drain, ~2 on
    others). Hoist extras onto preceding wait-only NoOps on the same engine."""
    import bass_rust
    for fn in nc.m.functions:
        for bb in fn.blocks:
            out = []
            for inst in bb.instructions:
                si = inst.sync_info
                waits = list(si.on_wait) if si is not None else []
                limit = 1
                if len(waits) > limit:
                    extra, keep = waits[:-limit], waits[-limit:]
                    for i in range(len(extra)):
                        nop = mybir.InstNoOp(
                            name=nc.get_next_instruction_name(), ins=[], outs=[])
                        nop.engine = inst.engine
                        nop.sync_info = bass_rust.SyncInfo(
                            on_wait=[extra[i]], on_update=[])
                        nc.register_instruction(nop)
                        out.append(nop)
                    inst.sync_info = bass_rust.SyncInfo(
                        on_wait=keep, on_update=list(si.on_update))
                out.append(inst)
            bb.instructions[:] = out


def shard_inputs(inputs):
    """Full inputs -> list of 8 per-core in_maps (numpy)."""
    import ml_dtypes
    BF = ml_dtypes.bfloat16
    e = np.ascontiguousarray(inputs["edge_fts"], dtype=np.float32)
    nf = np.ascontiguousarray(inputs["node_fts"], dtype=np.float32)
    hd = np.ascontiguousarray(inputs["hidden"], dtype=np.float32)
    gfa = np.ascontiguousarray(inputs["graph_fts"], dtype=np.float32)
    adj = np.asarray(inputs["adj_mat"])
    w = {k: np.ascontiguousarray(inputs[k], dtype=np.float32) for k in (
        "m_w", "m_b", "skip_w", "skip_b", "a1_w", "a1_b", "a2_w", "a2_b",
        "ae_w", "ae_b", "ag_w", "ag_b")}
    maps = []
    for c in range(NC):
        b, ih = c // 2, c % 2
        i0 = ih * NI
        # For odd cores, rotate the j axis (and z rows) by -512 so that the
        # core's own rows always sit at z columns 0..511. The attention sum
        # over j is permutation-invariant, so rolling e/adj/z consistently
        # leaves the output unchanged.
        ej = e[b, i0:i0 + NI]
        aj = adj[b, i0:i0 + NI, :]
        nfb, hdb = nf[b], hd[b]
        if ih == 1:
            ej = np.roll(ej, -NI, axis=1)
            aj = np.roll(aj, -NI, axis=1)
            nfb = np.roll(nfb, -NI, axis=0)
            hdb = np.roll(hdb, -NI, axis=0)
        # bf16 slabs per octet: e_own[oct][(j_lo,e)=128, (i8, j_hi)=1024]
        e_own = (ej.reshape(NI // 8, 8, 128, 8, 16)
                 .transpose(0, 3, 4, 1, 2).reshape(NI // 8, 128, 1024)
                 .astype(BF))
        # adjacency 0/1 mask per block: adjm[blk][j_hi=128, (i128, j_lo8)]
        adjm = (aj.reshape(NBLK, 128, 128, 8).transpose(0, 2, 1, 3)
                .reshape(NBLK, 128, 1024).astype(BF))
        m = {
            "e_own": np.ascontiguousarray(e_own),
            "nf": np.ascontiguousarray(nfb), "hd": np.ascontiguousarray(hdb),
            "gf": gfa[b].reshape(FG, 1),
            "adjm": np.ascontiguousarray(adjm),
            "m_w": w["m_w"], "m_b": w["m_b"].reshape(1, OUT),
            "skip_w": w["skip_w"], "skip_b": w["skip_b"].reshape(1, OUT),
            "a1_w": w["a1_w"], "a1_b": w["a1_b"].reshape(1, H),
            "a2_w": w["a2_w"], "a2_b": w["a2_b"].reshape(1, H),
            "ae_w": w["ae_w"], "ae_b": w["ae_b"].reshape(1, H),
            "ag_w": w["ag_w"], "ag_b": w["ag_b"].reshape(1, H),
        }
        maps.append(m)
    return maps


def build(n_iters=1):
    """One program shared by all 8 cores (inputs are pre-rotated so own
    rows always sit at z columns 0..511)."""
    nc = bass.Bass("TRN2", target_bir_lowering=False, debug=False,
                   num_devices=NC)
    build_core_program(nc, n_iters=n_iters)
    split_multi_waits(nc)
    return nc


def kernel(**inputs):
    from concourse.bass_utils import run_bass_kernel_spmd
    maps = shard_inputs(inputs)
    nc = build(n_iters=1)
    res = run_bass_kernel_spmd(nc, maps, list(range(NC))).results
    out = np.zeros((B, N, OUT), np.float32)
    for c in range(NC):
        b, ih = c // 2, c % 2
        out[b, ih * NI:(ih + 1) * NI] = res[c]["ret"]
    return out



# revision 25
# speedup vs baseline: 2.6119x; 2.6119x over previous
"""GAT layer Bass kernel for Trainium2, 8-core SPMD.

Sharding: core c handles batch b = c//2 and row-half ih = c%2 (512 rows of i).
Each core reads its 32MB slice of edge_fts once (memory-bound roofline).

Per-core layout trick: edge slab for row i is the contiguous 64KB region
E[i] viewed as [j_hi=128, (j_lo=8, e=16)=128].  A bf16 cast-load (SWDGE)
followed by an SBUF->SBUF DMA-transpose yields T_i[(j_lo,e), j_hi], and a
single matmul against a block-diagonal ae_w produces
att_e[j_hi, (h, j_lo)] with j (= j_hi*8+j_lo) on partitions -- the layout
softmax-over-j and the attention*V contraction both want.
"""
import sys
sys.path.insert(0, "/opt/trn_rl_repo")
from contextlib import ExitStack

import numpy as np

import concourse.bass as bass
import concourse.tile as tile
from concourse import mybir
from concourse.masks import make_identity

F32 = mybir.dt.float32
F32R = mybir.dt.float32r
BF16 = mybir.dt.bfloat16
AF = mybir.ActivationFunctionType
OP = mybir.AluOpType

B, N = 4, 1024
FN, FH, FE, FG = 128, 128, 16, 128
OUT, H = 128, 8
DH = OUT // H          # 16
ZIN = FN + FH          # 256
NC = 8                 # cores
NI = N // 2            # own rows per core = 512
NJH, NJL = N // 8, 8   # j = j_hi*8 + j_lo
NBLK = NI // 128       # i-blocks per core = 4
NOCT = 128 // 8        # octets per block = 16


def build_core_program(nc, n_iters=1):
    """Emit the full per-core program. Returns nothing; declares DRAM params:
    inputs: e_own[512,128,128]f32, nf[1024,128]f32, hd[1024,128]f32,
            gf[128,1]f32, adjb[128,4096]f32 (bias (adj-1)*1e9 in
            [j_hi, (i512, j_lo8)] layout),
            m_w[256,128], m_b[1,128], skip_w[256,128], skip_b[1,128],
            a1_w[256,8], a1_b[1,8], a2_w[256,8], a2_b[1,8],
            ae_w[16,8], ae_b[1,8], ag_w[128,8], ag_b[1,8]
    output: ret[512,128]f32
    """
    d = {}
    def inp(name, shape, dtype=F32):
        d[name] = nc.dram_tensor(name, shape, dtype, kind="ExternalInput").ap()
    inp("e_own", [NI // 8, 128, 1024], BF16)   # [oct, (j_lo,e), (i8, j_hi)]
    inp("nf", [N, FN])
    inp("hd", [N, FH])
    inp("gf", [FG, 1])
    inp("adjm", [NBLK, 128, 1024], BF16)       # [blk, j_hi, (i128, j_lo8)] 0/1
    inp("m_w", [ZIN, OUT]); inp("m_b", [1, OUT])
    inp("skip_w", [ZIN, OUT]); inp("skip_b", [1, OUT])
    inp("a1_w", [ZIN, H]); inp("a1_b", [1, H])
    inp("a2_w", [ZIN, H]); inp("a2_b", [1, H])
    inp("ae_w", [FE, H]); inp("ae_b", [1, H])
    inp("ag_w", [FG, H]); inp("ag_b", [1, H])
    ret = nc.dram_tensor("ret", [NI, OUT], F32, kind="ExternalOutput").ap()

    with tile.TileContext(nc) as tc:
        with ExitStack() as ctx:
            emit(ctx, tc, d, ret, n_iters)


def emit(ctx, tc, d, ret, n_iters):
    nc = tc.nc
    P = lambda name, bufs=1: ctx.enter_context(tc.tile_pool(name=name, bufs=bufs))
    PS = lambda name, bufs=1: ctx.enter_context(
        tc.tile_pool(name=name, bufs=bufs, space="PSUM"))

    const = P("const")          # long-lived constants / staging
    psc_pool = PS("ps_small", bufs=2)   # all [128, <=128] psum tiles, shared slots
    psmisc = PS("ps_misc", bufs=1)      # odd-shaped psum tiles
    class _PS:
        def tile(self, shape, dtype):
            return psc_pool.tile(shape, dtype, tag="s", name="pstile")
    psc = _PS()
    # ---------------- prologue: constants ----------------
    ident = const.tile([128, 128], F32)
    make_identity(nc, ident[:])
    ones_bf = const.tile([128, 128], BF16)
    nc.gpsimd.memset(ones_bf[:], 1.0)
    ones_row = const.tile([1, 128], F32)
    nc.gpsimd.memset(ones_row[:], 1.0)

    # small weights into sbuf; ZIN-row weights stored as [128, (2, n)]
    wpool = P("weights")
    def load(name, shape, dtype=F32):
        t = wpool.tile(shape, dtype, name=name)
        nc.gpsimd.dma_start(t[:], d[name][:])
        return t
    def load2(name, ncols):
        t = wpool.tile([128, 2, ncols], F32, name=name)
        nc.gpsimd.dma_start(t[:], d[name][:].rearrange("(c p) n -> p c n", c=2))
        return lambda ct: t[:, ct, :]
    m_w = load2("m_w", OUT);  m_b = load("m_b", [1, OUT])
    sk_w = load2("skip_w", OUT); sk_b = load("skip_b", [1, OUT])
    a1_w = load2("a1_w", H); a1_b = load("a1_b", [1, H])
    a2_w = load2("a2_w", H); a2_b = load("a2_b", [1, H])
    ae_w = load("ae_w", [FE, H]); ae_b = load("ae_b", [1, H])
    ag_w = load("ag_w", [FG, H]); ag_b = load("ag_b", [1, H])
    gf = load("gf", [FG, 1])
    import os
    K_STAGE = int(os.environ.get("K_STAGE", 99))
    if K_STAGE <= 1: return

    # blockdiag bd[(j_lo,e), (h, j_lo')] = ae_w[e,h] * (j_lo == j_lo')
    bd = const.tile([128, 64], BF16)
    nc.gpsimd.memset(bd[:], 0.0)
    ae_w_bf = wpool.tile([FE, H], BF16, name="ae_w_bf")
    nc.vector.tensor_copy(ae_w_bf[:], ae_w[:])
    for jl in range(NJL):
        # rows jl*16..jl*16+16, cols (h, jl) i.e. stride 8 offset jl
        dst = bd[:].rearrange("p (h j) -> p h j", h=H)[jl * 16:(jl + 1) * 16, :, jl]
        nc.gpsimd.dma_start(dst, ae_w_bf[:, :])

    if K_STAGE <= 2: return
    # I64 identity bf16 (for att2g selector matmul)
    i64 = const.tile([64, 64], BF16)
    make_identity(nc, i64[:])

    # zT: [c, j] two c-tiles of [128, 1024] f32
    zT = const.tile([128, 2 * N], F32)  # cols [0:1024] = nf.T, [1024:2048] = hd.T
    zpool = P("zstage", bufs=3)
    for half, src in ((0, d["nf"]), (1, d["hd"])):
        for jb in range(N // 128):
            st = zpool.tile([128, 128], F32)
            nc.gpsimd.dma_start(st[:], src[jb * 128:(jb + 1) * 128, :])
            tp = psc.tile([128, 128], F32)
            nc.tensor.transpose(tp[:], st[:], ident[:])
            nc.vector.tensor_copy(
                zT[:, half * N + jb * 128: half * N + (jb + 1) * 128], tp[:])

    if K_STAGE <= 3: return
    def zT_half(h_idx):
        return zT[:, h_idx * N:(h_idx + 1) * N]

    # cst[h] = a1_b + a2_b + ae_b + ag_b + gf @ ag_w   (shape [1, 8])
    attg_ps = psmisc.tile([1, H], F32, tag="m", name="attg_ps")
    nc.tensor.matmul(attg_ps[:], gf[:], ag_w[:],
                     start=True, stop=True)
    if K_STAGE <= 31: return
    cstv = const.tile([1, H], F32)
    nc.vector.scalar_tensor_tensor(cstv[:], a1_b[:], 1.0, a2_b[:], OP.mult, OP.add)
    nc.vector.scalar_tensor_tensor(cstv[:], cstv[:], 1.0, ae_b[:], OP.mult, OP.add)
    nc.vector.scalar_tensor_tensor(cstv[:], cstv[:], 1.0, ag_b[:], OP.mult, OP.add)
    nc.vector.scalar_tensor_tensor(cstv[:], cstv[:], 1.0, attg_ps[:], OP.mult, OP.add)
    if K_STAGE <= 32: return
    # broadcast cst to all 128 partitions: ones_row.T @ cstv
    cstb_ps = psmisc.tile([128, H], F32, tag="m", name="cstb_ps")
    nc.tensor.matmul(cstb_ps[:], ones_row[:], cstv[:],
                     start=True, stop=True)
    cstb = const.tile([128, H], F32)
    nc.vector.tensor_copy(cstb[:], cstb_ps[:])

    if K_STAGE <= 4: return
    # att2g[j_hi, (h, j_lo)] = att_2[j, h] + cst[h];  att2gT bf16 [64, 128]
    att2g = const.tile([128, 64], F32)
    for jl in range(NJL):
        a2ps = psc.tile([128, H], F32)
        for ct in range(2):
            lhs = zT_half(ct)[:].rearrange("p (j l) -> p j l", l=8)[:, :, jl]
            nc.tensor.matmul(a2ps[:], lhs,
                             a2_w(ct),
                             start=(ct == 0), stop=(ct == 1))
        dst = att2g[:].rearrange("p (h j) -> p h j", h=H)[:, :, jl]
        nc.vector.scalar_tensor_tensor(dst, a2ps[:], 1.0, cstb[:], OP.mult, OP.add)
    att2gT_ps = psc.tile([64, 128], F32)
    nc.tensor.transpose(att2gT_ps[:], att2g[:], ident[:])
    att2gT = const.tile([64, 128], BF16)
    nc.vector.tensor_copy(att2gT[:], att2gT_ps[:])

    if K_STAGE <= 5: return
    # q_sum[k, (h, i)] bf16: folded z (x) a1_w product so that
    # ones.T @ q_sum = att_1[h, i] broadcast over j_hi partitions.
    q_sum = const.tile([128, NI * H], BF16)
    qtmp = const.tile([128, NI * H], F32)
    # q = zT[c, own_i] * a1w[c, h]; own rows are always zT columns 0..511
    for ct in range(2):
        zslice = zT_half(ct)[:, OWN_I0:OWN_I0 + NI]
        z3 = zslice.rearrange("p (x i) -> p x i", x=1).broadcast_to([128, H, NI])
        a3 = a1_w(ct).rearrange("p (h x) -> p h x", x=1).broadcast_to([128, H, NI])
        if ct == 0:
            nc.vector.scalar_tensor_tensor(
                qtmp[:].rearrange("p (h i) -> p h i", h=H), z3, 1.0, a3,
                OP.mult, OP.mult)
        else:
            q2 = const.tile([128, NI * H], F32)
            nc.vector.scalar_tensor_tensor(
                q2[:].rearrange("p (h i) -> p h i", h=H), z3, 1.0, a3,
                OP.mult, OP.mult)
            nc.vector.scalar_tensor_tensor(
                q_sum[:].rearrange("p (h i) -> p h i", h=H),
                qtmp[:].rearrange("p (h i) -> p h i", h=H), 1.0,
                q2[:].rearrange("p (h i) -> p h i", h=H), OP.mult, OP.add)

    if K_STAGE <= 6: return
    # V_perm[j_hi, (h, j_lo, 17)] bf16; col 16 of each (h,j_lo) group is 1.0
    v_perm = const.tile([128, H * NJL * (DH + 1)], BF16)
    nc.gpsimd.memset(v_perm[:], 1.0)
    m_b_bc_ps = psc.tile([128, OUT], F32)
    nc.tensor.matmul(m_b_bc_ps[:], ones_row[:], m_b[:],
                     start=True, stop=True)
    m_b_bc = const.tile([128, OUT], F32)
    nc.vector.tensor_copy(m_b_bc[:], m_b_bc_ps[:])
    for jl in range(NJL):
        vps = psc.tile([128, OUT], F32)
        for ct in range(2):
            lhs = zT_half(ct)[:].rearrange("p (j l) -> p j l", l=8)[:, :, jl]
            nc.tensor.matmul(vps[:], lhs,
                             m_w(ct),
                             start=(ct == 0), stop=(ct == 1))
        dst = v_perm[:].rearrange("p (h j d) -> p h j d", h=H, j=NJL)[:, :, jl, 0:DH]
        nc.vector.scalar_tensor_tensor(
            dst, vps[:].rearrange("p (h d) -> p h d", h=H), 1.0,
            m_b_bc[:].rearrange("p (h d) -> p h d", h=H), OP.mult, OP.add)

    if K_STAGE <= 7: return
    # skip_b broadcast
    skb_ps = psc.tile([128, OUT], F32)
    nc.tensor.matmul(skb_ps[:], ones_row[:], sk_b[:],
                     start=True, stop=True)
    skb = const.tile([128, OUT], F32)
    nc.vector.tensor_copy(skb[:], skb_ps[:])

    # ---------------- main loop ----------------
    slabp = P("slab", bufs=6)
    maskp = P("maskb", bufs=2)
    lp = PS("logits", bufs=4)
    pblk = P("pblock", bufs=3)
    rp = P("rasm", bufs=2)
    outp = P("outs", bufs=2)

    import os
    nblk = int(os.environ.get("K_NBLK", NBLK))
    noct = int(os.environ.get("K_NOCT", NOCT))
    for it in range(n_iters):
        for ib in range(nblk):
            if DMA_ONLY:
                ret_probe = rp.tile([128, NOCT], F32, name="ret_probe")
            mblk = maskp.tile([128, 1024], BF16)
            nc.gpsimd.dma_start(mblk[:], d["adjm"][ib])
            p_block = pblk.tile([128, 128 * 64], BF16)  # (i 128, h 8, j_lo 8)
            for oct in range(noct):
                i0 = ib * 128 + oct * 8  # own-row index of first i in octet
                # bf16 slab, pre-transposed on host: [(j_lo,e), (i8, j_hi)]
                if not COMPUTE_ONLY or oct == 0:
                    s8 = slabp.tile([128, 1024], BF16)
                    nc.sync.dma_start(s8[:], d["e_own"][ib * NOCT + oct])
                if DMA_ONLY:
                    nc.vector.tensor_copy(
                        ret_probe[:, oct:oct + 1], s8[:, 0:1])
                    continue
                L = lp.tile([128, 512], F32)
                # col order: (h 8, j_lo 8, i 8)
                L4 = L[:].rearrange("p (h j i) -> p h j i", h=H, j=NJL)
                # att_1 (+ j_lo broadcast) : ones.T @ q_sum-slice
                qsl = (q_sum[:].rearrange("p (h x i) -> p h x i", x=1, i=NI)
                       [:, :, :, i0:i0 + 8].broadcast_to([128, H, NJL, 8]))
                nc.tensor.matmul(L4, ones_bf[:], qsl,
                                 start=True, stop=False, skip_group_check=True)
                # att_2 + cst : att2gT.T @ I64 (broadcast over i)
                isel = (i64[:].rearrange("p (n x) -> p n x", x=1)
                        .broadcast_to([64, 64, 8]))
                nc.tensor.matmul(L4.rearrange("p h j i -> p (h j) i"), att2gT[:],
                                 isel, start=False, stop=False, skip_group_check=True)
                # att_e per i
                for il in range(8):
                    lhs = s8[:, il * 128:(il + 1) * 128]
                    nc.tensor.matmul(L4[:, :, :, il], lhs,
                                     bd[:].rearrange("p (h j) -> p h j", h=H),
                                     start=False, stop=(il == 7),
                                     skip_group_check=True)
                # leaky relu in place (PSUM)
                nc.scalar.activation(L[:], L[:], AF.Prelu, alpha=0.01)
                # exp -> bf16 into P block (strided: block layout (h, j_lo, i128))
                pslice = (p_block[:].rearrange("p (h j i) -> p h j i", h=H, i=128)
                          [:, :, :, oct * 8:(oct + 1) * 8])
                nc.scalar.activation(pslice, L4, AF.Exp)
            # adjacency mask for whole block: multiply by 0/1 mask
            pv = p_block[:].rearrange("p (h f) -> p h f", h=H)
            mv = (mblk[:].rearrange("p (x f) -> p x f", x=1)
                  .broadcast_to([128, H, 1024]))
            nc.vector.tensor_tensor(pv, pv, mv, OP.mult)

            if DMA_ONLY:
                nc.gpsimd.dma_start(ret[ib * 128:(ib + 1) * 128, 0:NOCT],
                                    ret_probe[:])
                continue
            # attention @ V for this block
            r_asm = rp.tile([128, OUT], F32)
            pb4 = p_block[:].rearrange("p (h j i) -> p h j i", h=H, i=128)
            vp4 = v_perm[:].rearrange("p (h j d) -> p h j d", h=H, j=NJL)
            for h in range(H):
                av = psc.tile([128, DH + 1], F32)
                for jl in range(NJL):
                    nc.tensor.matmul(av[:], pb4[:, h, jl, :], vp4[:, h, jl, :],
                                     start=(jl == 0), stop=(jl == 7))
                recip = rp.tile([128, 1], F32)
                nc.vector.reciprocal(recip[:], av[:, DH:DH + 1])
                nc.vector.tensor_scalar_mul(
                    r_asm[:, h * DH:(h + 1) * DH], av[:, 0:DH], recip[:])

            # skip connection + relu + store
            sk = psc.tile([128, OUT], F32)
            for ct in range(2):
                lhs = zT_half(ct)[:, OWN_I0 + ib * 128:OWN_I0 + (ib + 1) * 128]
                nc.tensor.matmul(sk[:], lhs,
                                 sk_w(ct),
                                 start=(ct == 0), stop=False,
                                 skip_group_check=True)
            nc.tensor.matmul(sk[:], ones_row[:],
                             sk_b[:], start=False, stop=True,
                             skip_group_check=True)
            nc.vector.scalar_tensor_tensor(sk[:], sk[:], 1.0, r_asm[:],
                                           OP.mult, OP.add)
            ob = outp.tile([128, OUT], F32)
            nc.scalar.activation(ob[:], sk[:], AF.Relu)
            nc.gpsimd.dma_start(ret[ib * 128:(ib + 1) * 128, :], ob[:])


import os as _os
OWN_I0 = 0  # own rows always at z columns 0..511 (inputs pre-rotated)
USE_PRELU = True  # ACT parametric relu on HW; test_sim sets False
DMA_ONLY = _os.environ.get("K_DMA_ONLY", "0") == "1"
COMPUTE_ONLY = _os.environ.get("K_COMPUTE_ONLY", "0") == "1"


def split_multi_waits(nc):
    """Walrus codegen limits sem-waits per instruction (1 on Drain, ~2 on
    others). Hoist extras onto preceding wait-only NoOps on the same engine."""
    import bass_rust
    for fn in nc.m.functions:
        for bb in fn.blocks:
            out = []
            for inst in bb.instructions:
                si = inst.sync_info
                waits = list(si.on_wait) if si is not None else []
                limit = 1
                if len(waits) > limit:
                    extra, keep = waits[:-limit], waits[-limit:]
                    for i in range(len(extra)):
                        nop = mybir.InstNoOp(
                            name=nc.get_next_instruction_name(), ins=[], outs=[])
                        nop.engine = inst.engine
                        nop.sync_info = bass_rust.SyncInfo(
                            on_wait=[extra[i]], on_update=[])
                        nc.register_instruction(nop)
                        out.append(nop)
                    inst.sync_info = bass_rust.SyncInfo(
                        on_wait=keep, on_update=list(si.on_update))
                out.append(inst)
            bb.instructions[:] = out


def shard_inputs(inputs):
    """Full inputs -> list of 8 per-core in_maps (numpy)."""
    import ml_dtypes
    BF = ml_dtypes.bfloat16
    e = np.ascontiguousarray(inputs["edge_fts"], dtype=np.float32)
    nf = np.ascontiguousarray(inputs["node_fts"], dtype=np.float32)
    hd = np.ascontiguousarray(inputs["hidden"], dtype=np.float32)
    gfa = np.ascontiguousarray(inputs["graph_fts"], dtype=np.float32)
    adj = np.asarray(inputs["adj_mat"])
    w = {k: np.ascontiguousarray(inputs[k], dtype=np.float32) for k in (
        "m_w", "m_b", "skip_w", "skip_b", "a1_w", "a1_b", "a2_w", "a2_b",
        "ae_w", "ae_b", "ag_w", "ag_b")}
    maps = []
    for c in range(NC):
        b, ih = c // 2, c % 2
        i0 = ih * NI
        # For odd cores, rotate the j axis (and z rows) by -512 so that the
        # core's own rows always sit at z columns 0..511. The attention sum
        # over j is permutation-invariant, so rolling e/adj/z consistently
        # leaves the output unchanged.
        ej = e[b, i0:i0 + NI]
        aj = adj[b, i0:i0 + NI, :]
        nfb, hdb = nf[b], hd[b]
        if ih == 1:
            ej = np.roll(ej, -NI, axis=1)
            aj = np.roll(aj, -NI, axis=1)
            nfb = np.roll(nfb, -NI, axis=0)
            hdb = np.roll(hdb, -NI, axis=0)
        # bf16 slabs per octet: e_own[oct][(j_lo,e)=128, (i8, j_hi)=1024]
        e_own = (ej.reshape(NI // 8, 8, 128, 8, 16)
                 .transpose(0, 3, 4, 1, 2).reshape(NI // 8, 128, 1024)
                 .astype(BF))
        # adjacency 0/1 mask per block: adjm[blk][j_hi=128, (j_lo8, i128)]
        adjm = (aj.reshape(NBLK, 128, 128, 8).transpose(0, 2, 3, 1)
                .reshape(NBLK, 128, 1024).astype(BF))
        m = {
            "e_own": np.ascontiguousarray(e_own),
            "nf": np.ascontiguousarray(nfb), "hd": np.ascontiguousarray(hdb),
            "gf": gfa[b].reshape(FG, 1),
            "adjm": np.ascontiguousarray(adjm),
            "m_w": w["m_w"], "m_b": w["m_b"].reshape(1, OUT),
            "skip_w": w["skip_w"], "skip_b": w["skip_b"].reshape(1, OUT),
            "a1_w": w["a1_w"], "a1_b": w["a1_b"].reshape(1, H),
            "a2_w": w["a2_w"], "a2_b": w["a2_b"].reshape(1, H),
            "ae_w": w["ae_w"], "ae_b": w["ae_b"].reshape(1, H),
            "ag_w": w["ag_w"], "ag_b": w["ag_b"].reshape(1, H),
        }
        maps.append(m)
    return maps


def build(n_iters=1):
    """One program shared by all 8 cores (inputs are pre-rotated so own
    rows always sit at z columns 0..511)."""
    nc = bass.Bass("TRN2", target_bir_lowering=False, debug=False,
                   num_devices=NC)
    build_core_program(nc, n_iters=n_iters)
    split_multi_waits(nc)
    return nc


def kernel(**inputs):
    from concourse.bass_utils import run_bass_kernel_spmd
    maps = shard_inputs(inputs)
    nc = build(n_iters=1)
    res = run_bass_kernel_spmd(nc, maps, list(range(NC))).results
    out = np.zeros((B, N, OUT), np.float32)
    for c in range(NC):
        b, ih = c // 2, c % 2
        out[b, ih * NI:(ih + 1) * NI] = res[c]["ret"]
    return out

